# revision 1
# baseline (speedup 1.0000x reference)
"""GCN (3-layer, PyG GCNConv-style) + mean-pool + MLP head on 8 Trainium2 NeuronCores.

Strategy:
 - Nodes sharded by graph id (batch is sorted): B/8 graphs per core, contiguous
   node ranges, padded to NP nodes/core.
 - Edges sharded by dst owner; per core, edges are grouped into 128-dst-node
   "chunks" (CSR-style), and within each chunk bucketed by src range (int16
   dma_gather index limit), padded with -1 sentinels to a uniform capacity.
 - Per layer: z = h @ W computed locally per 128-node chunk, AllGather of the
   bf16 z table, dma_gather of per-edge src rows, segment-sum via one-hot
   matmuls on TensorE (edge norm folded into the selection matrix values).
 - Pooling via per-chunk graph-one-hot matmuls + AllReduce, tiny MLP head
   computed redundantly on every core.
"""
import sys
import numpy as np

sys.path.insert(0, "/opt/trn_rl_repo")

NCORES = 8
P = 128


def _roundup(x, m):
    return (x + m - 1) // m * m


def _wrap_idxs(v):
    """dma_gather idx layout: flat k -> [k%16, k//16], tiled to 128 partitions."""
    L = v.shape[0]
    return np.tile(v.reshape(L // 16, 16).T, (8, 1))


def _slotmajor(v):
    """Per-slot array [NSLOT] -> [128, NSLOT//128] with slot s -> [s%128, s//128]."""
    L = v.shape[0]
    return np.ascontiguousarray(v.reshape(L // 128, 128).T)


def preprocess(x, edge_index, batch, svm_pred, G=2):
    """All host-side layout work. Returns (params, in_maps, invc)."""
    import ml_dtypes
    bf16 = ml_dtypes.bfloat16

    N = x.shape[0]
    E = edge_index.shape[1]
    B = svm_pred.shape[0]
    H = 256
    gpc = B // NCORES  # graphs per core

    x = np.asarray(x, np.float32)
    ei = np.asarray(edge_index, np.int64)
    batch = np.asarray(batch, np.int64)

    # node ranges per core
    node_start = np.searchsorted(batch, np.arange(NCORES) * gpc)
    node_end = np.searchsorted(batch, np.arange(1, NCORES + 1) * gpc)
    cnts = node_end - node_start
    NC = int(_roundup(_roundup(cnts.max(), P) // P, G))
    NP = NC * P

    # full edge list with self loops
    loops = np.arange(N, dtype=np.int64)
    src = np.concatenate([ei[0], loops])
    dst = np.concatenate([ei[1], loops])
    deg = np.bincount(dst, minlength=N).astype(np.float32)
    dinv = deg ** -0.5
    norm = dinv[src] * dinv[dst]

    core_of = (batch // gpc).astype(np.int64)
    local_of = np.arange(N) - node_start[core_of]
    pg_of = core_of * NP + local_of  # padded-global node id

    # bucketing: two half-tables (locals [0,NP/2) and [NP/2,NP)), each split
    # into int16-range sub-buckets
    HNP = NP // 2
    RH = NCORES * HNP          # rows per half-table
    NBH = max(1, -(-RH // 32768))
    while RH % NBH:
        NBH += 1
    BS = RH // NBH
    assert BS <= 32768
    NBUCK = 2 * NBH

    dst_core = core_of[dst]
    src_half = (local_of // HNP).astype(np.int64)       # 0 or 1
    src_rowh = core_of * HNP + (local_of % HNP)          # row within half-table
    xg = x[src, 0] * norm  # layer-1 message values (norm folded in)

    # per (core, chunk, bucket) edge groups
    percore = []
    maxcnt = 0
    for c in range(NCORES):
        m = np.where(dst_core == c)[0]
        ldst = (dst[m] - node_start[c]).astype(np.int64)
        sh = src_half[src[m]]
        srh = src_rowh[src[m]]
        nb = sh * NBH + srh // BS
        srel = (srh - (srh // BS) * BS).astype(np.int16)
        chunk = ldst >> 7
        slot = (ldst & 127).astype(np.float32)
        key = chunk * NBUCK + nb
        order = np.argsort(key, kind="stable")
        key_s = key[order]
        bounds = np.searchsorted(key_s, np.arange(NC * NBUCK + 1))
        maxcnt = max(maxcnt, int(np.diff(bounds).max()))
        percore.append((order, bounds, srel, slot, xg[m].astype(np.float32)))

    BCAP = int(_roundup(max(maxcnt, 128), P))
    TPB = BCAP // P
    NG = NC // G
    NSLOT = NC * NBUCK * BCAP
    TT = NBUCK * TPB  # tiles per chunk

    gcnt = np.bincount(batch, minlength=B).astype(np.float32)
    invc = 1.0 / np.maximum(gcnt, 1.0)

    in_maps = []
    for c in range(NCORES):
        order, bounds, srel, slot, xgv = percore[c]
        idxf = np.zeros(NSLOT, np.int16)         # idx-0 pads (trim path desyncs the ring)
        slotf = np.full(NSLOT, -1.0, np.float32)  # -1 => all-zero one-hot row
        xgf = np.zeros(NSLOT, np.float32)
        for ch in range(NC):
            g, j = ch // G, ch % G
            for b in range(NBUCK):
                k = ch * NBUCK + b
                lo, hi = bounds[k], bounds[k + 1]
                n = hi - lo
                sel = order[lo:hi]
                cbase = ((g * NBUCK + b) * G + j) * BCAP  # call-major
                idxf[cbase:cbase + n] = srel[sel]
                mbase = (ch * NBUCK + b) * BCAP            # chunk-major
                slotf[mbase:mbase + n] = slot[sel]
                xgf[mbase:mbase + n] = xgv[sel]

        # wrapped idx per call: call (g,b) covers [call*G*BCAP, (call+1)*G*BCAP)
        CW = G * BCAP // 16
        idxw = np.empty((P, NG * NBUCK * CW), np.int16)
        for call in range(NG * NBUCK):
            seg = idxf[call * G * BCAP:(call + 1) * G * BCAP]
            idxw[:, call * CW:(call + 1) * CW] = _wrap_idxs(seg)

        gidl = np.full(NP, -1.0, np.float32)
        n = cnts[c]
        gidl[:n] = batch[node_start[c]:node_end[c]].astype(np.float32)
        dvl = np.zeros(NP, np.float32)
        dvl[:n] = dinv[node_start[c]:node_end[c]]

        import ml_dtypes as _md
        slotm = _slotmajor(slotf)  # [128, NSLOT//128]
        p01 = (slotm[:, :, None] == np.arange(P, dtype=np.float32)).astype(_md.float8_e4m3)
        in_maps.append({
            "idxw": idxw,
            "p01": np.ascontiguousarray(p01.reshape(P, (NSLOT // P) * P)),
            "xgv": _slotmajor(xgf).astype(bf16),
            "gid": np.ascontiguousarray(gidl.reshape(NC, P).T),
            "dinvl": np.ascontiguousarray(dvl.reshape(NC, P).T),
        })

    params = dict(N=N, E=E, B=B, H=H, NP=NP, NC=NC, NBUCK=NBUCK, BS=BS,
                  BCAP=BCAP, TPB=TPB, G=G, NG=NG, NSLOT=NSLOT, TT=TT, NBH=NBH)
    return params, in_maps, invc


def add_weight_inputs(in_maps, params, W1, b1, W2, b2, W3, b3, Wf1, bf1, Wf2, bf2,
                      svm_pred, invc):
    import ml_dtypes
    bf16 = ml_dtypes.bfloat16
    B, H = params["B"], params["H"]
    f32 = np.float32

    def kswiz(W, width):  # [256, width] -> [128, 2*width] with [p, k*width+h]
        W = np.asarray(W, f32)
        return np.ascontiguousarray(
            W.reshape(2, P, width).transpose(1, 0, 2).reshape(P, 2 * width))

    shared = {
        "W1rep": np.repeat(np.asarray(W1, f32).reshape(1, H), P, 0),
        "b1rep": np.repeat(np.asarray(b1, f32).reshape(1, H), P, 0),
        "W2s": kswiz(W2, H).astype(bf16),
        "W3s": kswiz(W3, H).astype(bf16),
        "b2rep": np.repeat(np.asarray(b2, f32).reshape(1, H), P, 0),
        "b3rep": np.repeat(np.asarray(b3, f32).reshape(1, H), P, 0),
        "Wf1k": kswiz(np.asarray(Wf1, f32)[:2 * P], 128),
        "Wf1c": np.ascontiguousarray(np.asarray(Wf1, f32)[2 * P:].reshape(1, 128)),
        "bf1rep": np.repeat(np.asarray(bf1, f32).reshape(1, 128), B, 0),
        "Wf2s": np.asarray(Wf2, f32).reshape(P, 6),
        "bf2rep": np.repeat(np.asarray(bf2, f32).reshape(1, 6), B, 0),
        "svm": np.asarray(svm_pred, f32).reshape(1, B),
        "invc2": np.tile(np.asarray(invc, f32).reshape(1, 1, B),
                         (P, 2, 1)).reshape(P, 2 * B),
    }
    for m in in_maps:
        m.update(shared)


def build(params, cut=0):
    import concourse.bass as bass
    import concourse.bacc as bacc
    import concourse.tile as tile
    from concourse import mybir
    from concourse.masks import make_identity

    NP, NC, NBUCK, BS = params["NP"], params["NC"], params["NBUCK"], params["BS"]
    NBH, HNP = params["NBH"], params["NP"] // 2
    RH = NCORES * HNP
    BCAP, TPB, G, NG = params["BCAP"], params["TPB"], params["G"], params["NG"]
    NSLOT, TT, B, H = params["NSLOT"], params["TT"], params["B"], params["H"]
    TABLE = NCORES * NP
    CW = G * BCAP // 16
    GT = NBUCK * G * TPB  # msgs staging tiles per group

    FT = mybir.dt.float32
    BF = mybir.dt.bfloat16
    ZD = mybir.dt.float8e4 if params.get("zfp8") else mybir.dt.bfloat16
    I16 = mybir.dt.int16
    AL = mybir.AluOpType
    AF = mybir.ActivationFunctionType

    nc = bacc.Bacc("TRN2", target_bir_lowering=False, debug=False,
                   num_devices=NCORES, num_swdge_queues=4)

    dp = nc.declare_dram_parameter
    pr = {
        "idxw": dp("idxw", [P, NG * NBUCK * CW], I16, isOutput=False),
        "p01": dp("p01", [P, (NSLOT // P) * P], mybir.dt.float8e4, isOutput=False),
        "xgv": dp("xgv", [P, NSLOT // P], BF, isOutput=False),
        "gid": dp("gid", [P, NC], FT, isOutput=False),
        "dinvl": dp("dinvl", [P, NC], FT, isOutput=False),
        "W1rep": dp("W1rep", [P, H], FT, isOutput=False),
        "b1rep": dp("b1rep", [P, H], FT, isOutput=False),
        "W2s": dp("W2s", [P, 2 * H], BF, isOutput=False),
        "W3s": dp("W3s", [P, 2 * H], BF, isOutput=False),
        "b2rep": dp("b2rep", [P, H], FT, isOutput=False),
        "b3rep": dp("b3rep", [P, H], FT, isOutput=False),
        "Wf1k": dp("Wf1k", [P, 2 * 128], FT, isOutput=False),
        "Wf1c": dp("Wf1c", [1, 128], FT, isOutput=False),
        "bf1rep": dp("bf1rep", [B, 128], FT, isOutput=False),
        "Wf2s": dp("Wf2s", [P, 6], FT, isOutput=False),
        "bf2rep": dp("bf2rep", [B, 6], FT, isOutput=False),
        "svm": dp("svm", [1, B], FT, isOutput=False),
        "invc2": dp("invc2", [P, 2 * B], FT, isOutput=False),
    }
    out_p = dp("out", [B, 6], FT, isOutput=True)

    with tile.TileContext(nc) as tc:
        with (
            tc.tile_pool(name="res", bufs=1) as res,
            tc.tile_pool(name="work", bufs=3) as work,
            tc.tile_pool(name="pp_acc", bufs=2, space="PSUM") as pp_acc,
            tc.tile_pool(name="pp_z", bufs=2, space="PSUM") as pp_z,
            tc.tile_pool(name="pp_t", bufs=2, space="PSUM") as pp_t,
            tc.tile_pool(name="pp_pool", bufs=1, space="PSUM") as pp_pool,
            tc.tile_pool(name="dram", bufs=1, space="DRAM") as dram,
        ):
            zloc = dram.tile([NP, H], ZD, name="zloc")
            ztab2A = dram.tile([RH, H], ZD, name="ztab2A")
            ztab2B = dram.tile([RH, H], ZD, name="ztab2B")
            ztab3A = dram.tile([RH, H], ZD, name="ztab3A")
            ztab3B = dram.tile([RH, H], ZD, name="ztab3B")
            ccin = dram.tile([P, 2 * B], FT, name="ccin")
            ccout = dram.tile([P, 2 * B], FT, addr_space="Shared", name="ccout")

            # ---- resident SBUF ----
            sizes = {
                "idxw": ([P, NG * NBUCK * CW], I16),
                "xgv": ([P, NSLOT // P], BF),
                "gid": ([P, NC], FT),
                "dinvl": ([P, NC], FT),
                "W1rep": ([P, H], FT),
                "b1rep": ([P, H], FT),
                "W2s": ([P, 2 * H], BF),
                "W3s": ([P, 2 * H], BF),
                "b2rep": ([P, H], FT),
                "b3rep": ([P, H], FT),
                "Wf1k": ([P, 2 * 128], FT),
                "Wf1c": ([1, 128], FT),
                "bf1rep": ([B, 128], FT),
                "Wf2s": ([P, 6], FT),
                "bf2rep": ([B, 6], FT),
                "svm": ([1, B], FT),
                "invc2": ([P, 2 * B], FT),
            }
            sb = {}
            for k, (shape, dt) in sizes.items():
                sb[k] = res.tile(shape, dt, name=f"sb_{k}")
                nc.sync.dma_start(sb[k][:], pr[k][:])
            p01_dram = pr["p01"]

            iota128 = res.tile([P, P], BF)
            nc.gpsimd.iota(iota128[:], pattern=[[1, P]], base=0,
                           channel_multiplier=0, allow_small_or_imprecise_dtypes=True)
            iota64 = res.tile([P, B], FT)
            nc.gpsimd.iota(iota64[:], pattern=[[1, B]], base=0,
                           channel_multiplier=0, allow_small_or_imprecise_dtypes=True)
            ident = res.tile([P, P], FT)
            make_identity(nc, ident[:])

            msgs = [res.tile([P, GT, H], ZD, name=f"msgs{i}") for i in range(3)]
            for _m in msgs:
                nc.vector.memset(_m[:], 0.0)

            def build_P(ch):
                """Stream host-precomputed 0/1 one-hot [128 edges, TT*128 dst]."""
                Pt = work.tile([P, TT * P], mybir.dt.float8e4, tag="P")
                lo = ch * TT * P
                nc.sync.dma_start(Pt[:], p01_dram[:, lo:lo + TT * P])
                return Pt

            def h_to_z(hc, W_sb, ch):
                """z chunk = dinv * (h chunk @ W) -> bf16 rows of zloc."""
                hT = work.tile([P, 2, P], BF, tag="hT")
                for k in range(2):
                    tp = pp_t.tile([P, P], FT, tag="tp", space="PSUM")
                    nc.tensor.transpose(out=tp[:], in_=hc[:, k * P:(k + 1) * P],
                                        identity=ident[:])
                    nc.vector.tensor_copy(hT[:, k, :], tp[:])
                zp = pp_z.tile([P, H], FT, tag="zp", space="PSUM")
                for k in range(2):
                    nc.tensor.matmul(zp[:], lhsT=hT[:, k, :],
                                     rhs=W_sb[:, k * H:(k + 1) * H],
                                     start=(k == 0), stop=(k == 1))
                zb = work.tile([P, H], ZD, tag="zb")
                nc.vector.tensor_scalar(out=zb[:], in0=zp[:],
                                        scalar1=sb["dinvl"][:, ch:ch + 1],
                                        scalar2=None, op0=AL.mult)
                nc.sync.dma_start(zloc[ch * P:(ch + 1) * P, :], zb[:])

            def emit_ag(half, tabA, tabB):
                if half == 0:
                    nc.gpsimd.collective_compute(
                        "AllGather", AL.bypass,
                        replica_groups=[list(range(NCORES))],
                        ins=[zloc[0:HNP, :]], outs=[tabA.opt()])
                else:
                    nc.gpsimd.collective_compute(
                        "AllGather", AL.bypass,
                        replica_groups=[list(range(NCORES))],
                        ins=[zloc[HNP:NP, :]], outs=[tabB.opt()])

            # ================= LAYER 1 (no gather; xg resident) =================
            for ch in range(NC):
                Pt = build_P(ch)
                sT = pp_acc.tile([P, H], FT, tag="acc", space="PSUM")
                for i in range(TT):
                    gt2 = ch * TT + i
                    nc.tensor.matmul(sT[0:1, 0:P], lhsT=sb["xgv"][:, gt2:gt2 + 1],
                                     rhs=Pt[:, i * P:(i + 1) * P],
                                     start=(i == 0), stop=(i == TT - 1))
                sTr = work.tile([1, P], FT, tag="sTr")
                nc.vector.tensor_copy(sTr[:], sT[0:1, 0:P])
                s1 = pp_t.tile([P, P], FT, tag="tp", space="PSUM")
                nc.tensor.transpose(out=s1[:, 0:1], in_=sTr[:], identity=ident[0:1, 0:1])
                h1 = work.tile([P, H], FT, tag="hc")
                nc.vector.scalar_tensor_tensor(
                    out=h1[:], in0=sb["W1rep"][:], scalar=s1[:, 0:1],
                    in1=sb["b1rep"][:], op0=AL.mult, op1=AL.add)
                nc.scalar.activation(out=h1[:], in_=h1[:], func=AF.Relu)
                h_to_z(h1, sb["W2s"], ch)
                if ch == NC // 2 - 1:
                    emit_ag(0, ztab2A, ztab2B)
            emit_ag(1, ztab2A, ztab2B)

            if cut == 1:
                fin0 = work.tile([B, 6], FT, tag="fin")
                nc.vector.memset(fin0[:], 0.0)
                nc.sync.dma_start(out_p[:], fin0[:])
            # ================= LAYERS 2,3 =================
            poolTs = [pp_pool.tile([P, B], FT, tag=f"pool{k}", space="PSUM", name=f"poolT{k}") for k in range(2)]

            def gather_group(g, tabs, pingpong):
                mt = msgs[pingpong % len(msgs)]
                for b in range(NBUCK):
                    call = g * NBUCK + b
                    tab = tabs[b // NBH]
                    sub = b % NBH
                    nc.gpsimd.dma_gather(
                        mt[:, b * G * TPB:(b + 1) * G * TPB, :],
                        tab[sub * BS:(sub + 1) * BS, :],
                        sb["idxw"][:, call * CW:(call + 1) * CW],
                        G * BCAP, G * BCAP, H, single_packet=False,
                        queue_num=b % 4)
                return mt

            def msg_layer(tabs, W_next, brow, is_last, sub=4, next_tabs=None):
                for g in range(NG):
                    mt = gather_group(g, tabs, g % len(msgs))
                    if sub < 2:
                        continue
                    for j in range(G):
                        ch = g * G + j
                        Pt = build_P(ch)
                        acc = pp_acc.tile([P, H], FT, tag="acc", space="PSUM")
                        i = 0
                        for b in range(NBUCK):
                            for t in range(TPB):
                                nc.tensor.matmul(
                                    acc[:], lhsT=Pt[:, i * P:(i + 1) * P],
                                    rhs=mt[:, (b * G + j) * TPB + t, :],
                                    start=(i == 0),
                                    stop=(i == NBUCK * TPB - 1))
                                i += 1
                        if sub == 2:
                            continue
                        hc = work.tile([P, H], FT, tag="hc")
                        if brow is None:
                            nc.scalar.activation(out=hc[:], in_=acc[:], func=AF.Relu,
                                                 scale=sb["dinvl"][:, ch:ch + 1])
                        else:
                            nc.vector.scalar_tensor_tensor(
                                out=hc[:], in0=acc[:],
                                scalar=sb["dinvl"][:, ch:ch + 1],
                                in1=brow[:], op0=AL.mult, op1=AL.add)
                            nc.scalar.activation(out=hc[:], in_=hc[:], func=AF.Relu)
                        if sub == 3:
                            continue
                        if not is_last:
                            h_to_z(hc, W_next, ch)
                            if ch == NC // 2 - 1 and next_tabs is not None:
                                emit_ag(0, *next_tabs)
                        else:
                            oh = work.tile([P, B], FT, tag="oh")
                            nc.vector.tensor_scalar(
                                out=oh[:], in0=iota64[:],
                                scalar1=sb["gid"][:, ch:ch + 1], scalar2=None,
                                op0=AL.is_equal)
                            for k in range(2):
                                nc.tensor.matmul(
                                    poolTs[k][:], lhsT=hc[:, k * P:(k + 1) * P],
                                    rhs=oh[:], start=(ch == 0), stop=(ch == NC - 1))

            if cut != 1:
                msg_layer((ztab2A, ztab2B), sb["W3s"],
                          None if params.get("b2z") else sb["b2rep"], False,
                          sub=(cut - 20 if 20 < cut < 25 else 4),
                          next_tabs=(ztab3A, ztab3B))
                emit_ag(1, ztab3A, ztab3B)
            if cut == 2 or 20 < cut < 25:
                fin0 = work.tile([B, 6], FT, tag="fin")
                nc.vector.memset(fin0[:], 0.0)
                nc.sync.dma_start(out_p[:], fin0[:])
            if cut == 0:
                msg_layer((ztab3A, ztab3B), None,
                          None if params.get("b3z") else sb["b3rep"], True)

            if cut == 0:
                # ---- pool epilogue ----
                poolsb = work.tile([P, 2 * B], FT, tag="poolsb")
                for k in range(2):
                    nc.vector.tensor_copy(poolsb[:, k * B:(k + 1) * B], poolTs[k][:])
                nc.sync.dma_start(ccin[:], poolsb[:])
                nc.gpsimd.collective_compute(
                    "AllReduce", AL.add, replica_groups=[list(range(NCORES))],
                    ins=[ccin.opt()], outs=[ccout.opt()])
                pooledT = work.tile([P, 2 * B], FT, tag="pooledT")
                nc.sync.dma_start(pooledT[:], ccout[:])
                nc.vector.tensor_tensor(out=pooledT[:], in0=pooledT[:],
                                        in1=sb["invc2"][:], op=AL.mult)

                # ---- MLP head ----
                o1 = pp_acc.tile([B, 128], FT, tag="acc", space="PSUM")
                pT = pooledT[:].rearrange("p (k b) -> p k b", k=2)
                for k in range(2):
                    nc.tensor.matmul(o1[:], lhsT=pT[:, k, :],
                                     rhs=sb["Wf1k"][:, k * 128:(k + 1) * 128],
                                     start=(k == 0), stop=False)
                nc.tensor.matmul(o1[:], lhsT=sb["svm"][:], rhs=sb["Wf1c"][:],
                                 start=False, stop=True)
                a1 = work.tile([B, 128], FT, tag="a1")
                nc.vector.scalar_tensor_tensor(out=a1[:], in0=o1[:], scalar=1.0,
                                               in1=sb["bf1rep"][:], op0=AL.mult,
                                               op1=AL.add)
                nc.scalar.activation(out=a1[:], in_=a1[:], func=AF.Relu)
                tpa = pp_t.tile([P, B], FT, tag="tp", space="PSUM")
                nc.tensor.transpose(out=tpa[:], in_=a1[:], identity=ident[0:B, 0:B])
                a1T = work.tile([P, B], FT, tag="a1T")
                nc.vector.tensor_copy(a1T[:], tpa[:])
                o2 = pp_z.tile([B, 6], FT, tag="zp", space="PSUM")
                nc.tensor.matmul(o2[:], lhsT=a1T[:], rhs=sb["Wf2s"][:],
                                 start=True, stop=True)
                fin = work.tile([B, 6], FT, tag="fin")
                nc.vector.scalar_tensor_tensor(out=fin[:], in0=o2[:], scalar=1.0,
                                               in1=sb["bf2rep"][:], op0=AL.mult,
                                               op1=AL.add)
                nc.sync.dma_start(out_p[:], fin[:])

    nc.compile()
    return nc


def kernel(x, edge_index, batch, svm_pred,
           W1, b1, W2, b2, W3, b3, Wf1, bf1, Wf2, bf2, **kw):
    from concourse.bass_utils import run_bass_kernel_spmd
    params, in_maps, invc = preprocess(x, edge_index, batch, svm_pred)
    add_weight_inputs(in_maps, params, W1, b1, W2, b2, W3, b3, Wf1, bf1, Wf2, bf2,
                      svm_pred, invc)
    params["b2z"] = not np.any(np.asarray(b2))
    params["b3z"] = not np.any(np.asarray(b3))
    params["zfp8"] = True
    nc = build(params)
    res = run_bass_kernel_spmd(nc, in_maps, core_ids=list(range(NCORES)), **kw)
    out = np.asarray(res.results[0]["out"], np.float32)
    if kw:
        return out, res
    return out



# revision 2
# speedup vs baseline: 1.0104x; 1.0104x over previous
"""GCN (3-layer, PyG GCNConv-style) + mean-pool + MLP head on 8 Trainium2 NeuronCores.

v2 strategy (per-core, nodes sharded by graph id; batch sorted):
 - Self-loops removed from the gathered edge list: the self term is added from
   an SBUF-resident copy of the local z chunk (and via a host-shipped x*dinv^2
   column for layer 1's scalar aggregation).
 - Edges bucketed by src-local QUARTER (4 tables of 25600 rows, int16-safe).
   Each quarter table is AllGather'd independently as soon as the producing
   quarter of z is done -> 4 pipelined AGs per layer instead of 2.
 - BCAP (slot capacity per chunk x bucket) sized from the no-loop max (640 vs
   768) -> ~17% fewer dma_gather descriptors, P bytes, and matmul tiles.
 - Layer 1 uses swapped-operand matmuls (lhsT = one-hot P tile, rhs = edge
   scalar column) accumulating s[128,1] per chunk directly, and (when b1 == 0)
   the rank-2 identity relu(s*W1) @ W2 = relu(s) u + relu(-s) v to skip h1 and
   h_to_z entirely.
 - Aggregation per 128-dst chunk via one-hot fp8 matmuls on TensorE; msgs
   staged fp8; pooling via graph-one-hot matmuls + AllReduce; MLP head
   computed redundantly per core.
"""
import sys
import numpy as np

sys.path.insert(0, "/opt/trn_rl_repo")

NCORES = 8
P = 128
NBUCK = 4


def _roundup(x, m):
    return (x + m - 1) // m * m


def _wrap_idxs(v):
    """dma_gather idx layout: flat k -> [k%16, k//16], tiled to 128 partitions."""
    L = v.shape[0]
    return np.tile(v.reshape(L // 16, 16).T, (8, 1))


def _slotmajor(v):
    """Per-slot array [NSLOT] -> [128, NSLOT//128] with slot s -> [s%128, s//128]."""
    L = v.shape[0]
    return np.ascontiguousarray(v.reshape(L // 128, 128).T)


def preprocess(x, edge_index, batch, svm_pred, G=2):
    """All host-side layout work. Returns (params, in_maps, invc)."""
    import ml_dtypes
    bf16 = ml_dtypes.bfloat16

    N = x.shape[0]
    E = edge_index.shape[1]
    B = svm_pred.shape[0]
    H = 256
    gpc = B // NCORES  # graphs per core

    x = np.asarray(x, np.float32)
    ei = np.asarray(edge_index, np.int64)
    batch = np.asarray(batch, np.int64)

    # node ranges per core
    node_start = np.searchsorted(batch, np.arange(NCORES) * gpc)
    node_end = np.searchsorted(batch, np.arange(1, NCORES + 1) * gpc)
    cnts = node_end - node_start
    NC = int(_roundup(_roundup(cnts.max(), P) // P, G))
    if NC % NBUCK:
        NC = int(_roundup(NC, G * NBUCK))
    NP = NC * P
    QS = NP // NBUCK          # local rows per quarter table
    RQ = NCORES * QS          # rows per quarter table
    assert RQ <= 32768, RQ    # int16 idx limit
    assert NC % NBUCK == 0 and NC % G == 0

    # real edges only; deg includes the self-loop (+1)
    src, dst = ei[0], ei[1]
    deg = (np.bincount(dst, minlength=N) + 1.0).astype(np.float32)
    dinv = deg ** -0.5
    norm = dinv[src] * dinv[dst]

    core_of = (batch // gpc).astype(np.int64)
    local_of = np.arange(N) - node_start[core_of]

    src_q = local_of // QS                       # quarter of src
    src_row = core_of * QS + (local_of % QS)     # row within quarter table
    xg = x[:, 0][src] * norm                     # layer-1 edge values
    xself = x[:, 0] * dinv * dinv                # layer-1 self term

    # per (core, chunk, bucket) edge groups
    percore = []
    maxcnt = 0
    for c in range(NCORES):
        m = np.where(core_of[dst] == c)[0]
        ldst = (dst[m] - node_start[c]).astype(np.int64)
        nb = src_q[src[m]]
        srel = src_row[src[m]].astype(np.int16)
        chunk = ldst >> 7
        slot = (ldst & 127).astype(np.float32)
        key = chunk * NBUCK + nb
        order = np.argsort(key, kind="stable")
        key_s = key[order]
        bounds = np.searchsorted(key_s, np.arange(NC * NBUCK + 1))
        maxcnt = max(maxcnt, int(np.diff(bounds).max()))
        percore.append((order, bounds, srel, slot, xg[m].astype(np.float32)))

    BCAP = int(_roundup(max(maxcnt, 128), P))
    TPB = BCAP // P
    NG = NC // G
    NSLOT = NC * NBUCK * BCAP
    TT = NBUCK * TPB  # tiles per chunk

    gcnt = np.bincount(batch, minlength=B).astype(np.float32)
    invc = 1.0 / np.maximum(gcnt, 1.0)

    in_maps = []
    for c in range(NCORES):
        order, bounds, srel, slot, xgv = percore[c]
        idxf = np.zeros(NSLOT, np.int16)          # idx-0 pads (trim desyncs ring)
        slotf = np.full(NSLOT, -1.0, np.float32)  # -1 => all-zero one-hot row
        xgf = np.zeros(NSLOT, np.float32)
        for ch in range(NC):
            g, j = ch // G, ch % G
            for b in range(NBUCK):
                k = ch * NBUCK + b
                lo, hi = bounds[k], bounds[k + 1]
                n = hi - lo
                sel = order[lo:hi]
                cbase = ((g * NBUCK + b) * G + j) * BCAP  # call-major
                idxf[cbase:cbase + n] = srel[sel]
                mbase = (ch * NBUCK + b) * BCAP            # chunk-major
                slotf[mbase:mbase + n] = slot[sel]
                xgf[mbase:mbase + n] = xgv[sel]

        # wrapped idx per call: call (g,b) covers [call*G*BCAP, (call+1)*G*BCAP)
        CW = G * BCAP // 16
        idxw = np.empty((P, NG * NBUCK * CW), np.int16)
        for call in range(NG * NBUCK):
            seg = idxf[call * G * BCAP:(call + 1) * G * BCAP]
            idxw[:, call * CW:(call + 1) * CW] = _wrap_idxs(seg)

        n = cnts[c]
        gidl = np.full(NP, -1.0, np.float32)
        gidl[:n] = batch[node_start[c]:node_end[c]].astype(np.float32)
        dvl = np.zeros(NP, np.float32)
        dvl[:n] = dinv[node_start[c]:node_end[c]]
        xsl = np.zeros(NP, np.float32)
        xsl[:n] = xself[node_start[c]:node_end[c]]

        slotm = _slotmajor(slotf)  # [128, NSLOT//128]
        p01 = (slotm[:, :, None] == np.arange(P, dtype=np.float32)).astype(
            ml_dtypes.float8_e4m3)
        in_maps.append({
            "idxw": idxw,
            "p01": np.ascontiguousarray(p01.reshape(P, (NSLOT // P) * P)),
            "xgv": _slotmajor(xgf).astype(bf16),
            "gid": np.ascontiguousarray(gidl.reshape(NC, P).T),
            "dinvl": np.ascontiguousarray(dvl.reshape(NC, P).T),
            "xsf": np.ascontiguousarray(xsl.reshape(NC, P).T),
            "xsfn": np.ascontiguousarray((-xsl).reshape(NC, P).T),
        })

    params = dict(N=N, E=E, B=B, H=H, NP=NP, NC=NC, QS=QS, RQ=RQ,
                  BCAP=BCAP, TPB=TPB, G=G, NG=NG, NSLOT=NSLOT, TT=TT)
    return params, in_maps, invc


def add_weight_inputs(in_maps, params, W1, b1, W2, b2, W3, b3, Wf1, bf1, Wf2, bf2,
                      svm_pred, invc):
    import ml_dtypes
    bf16 = ml_dtypes.bfloat16
    B, H = params["B"], params["H"]
    f32 = np.float32

    def kswiz(W, width):  # [256, width] -> [128, 2*width] with [p, k*width+h]
        W = np.asarray(W, f32)
        return np.ascontiguousarray(
            W.reshape(2, P, width).transpose(1, 0, 2).reshape(P, 2 * width))

    W1r = np.asarray(W1, f32).reshape(H)
    u = np.maximum(W1r, 0.0) @ np.asarray(W2, f32)     # [H]
    v = np.maximum(-W1r, 0.0) @ np.asarray(W2, f32)    # [H]

    shared = {
        "urep": np.repeat(u.reshape(1, H), P, 0),
        "vrep": np.repeat(v.reshape(1, H), P, 0),
        "W1rep": np.repeat(np.asarray(W1, f32).reshape(1, H), P, 0),
        "b1rep": np.repeat(np.asarray(b1, f32).reshape(1, H), P, 0),
        "W2s": kswiz(W2, H).astype(bf16),
        "W3s": kswiz(W3, H).astype(bf16),
        "b2rep": np.repeat(np.asarray(b2, f32).reshape(1, H), P, 0),
        "b3rep": np.repeat(np.asarray(b3, f32).reshape(1, H), P, 0),
        "Wf1k": kswiz(np.asarray(Wf1, f32)[:2 * P], 128),
        "Wf1c": np.ascontiguousarray(np.asarray(Wf1, f32)[2 * P:].reshape(1, 128)),
        "bf1rep": np.repeat(np.asarray(bf1, f32).reshape(1, 128), B, 0),
        "Wf2s": np.asarray(Wf2, f32).reshape(P, 6),
        "bf2rep": np.repeat(np.asarray(bf2, f32).reshape(1, 6), B, 0),
        "svm": np.asarray(svm_pred, f32).reshape(1, B),
        "invc2": np.tile(np.asarray(invc, f32).reshape(1, 1, B),
                         (P, 2, 1)).reshape(P, 2 * B),
    }
    for m in in_maps:
        m.update(shared)


def build(params, cut=0):
    import concourse.bacc as bacc
    import concourse.tile as tile
    from concourse import mybir
    from concourse.masks import make_identity

    NP, NC, QS, RQ = params["NP"], params["NC"], params["QS"], params["RQ"]
    BCAP, TPB, G, NG = params["BCAP"], params["TPB"], params["G"], params["NG"]
    NSLOT, TT, B, H = params["NSLOT"], params["TT"], params["B"], params["H"]
    QC = NC // NBUCK          # chunks per quarter
    CW = G * BCAP // 16
    GT = NBUCK * G * TPB      # msgs staging tiles per group
    l1_fast = bool(params.get("l1_fast", True))

    FT = mybir.dt.float32
    BF = mybir.dt.bfloat16
    F8 = mybir.dt.float8e4
    I16 = mybir.dt.int16
    AL = mybir.AluOpType
    AF = mybir.ActivationFunctionType

    nc = bacc.Bacc("TRN2", target_bir_lowering=False, debug=False,
                   num_devices=NCORES, num_swdge_queues=4)

    dp = nc.declare_dram_parameter
    pr = {
        "idxw": dp("idxw", [P, NG * NBUCK * CW], I16, isOutput=False),
        "p01": dp("p01", [P, (NSLOT // P) * P], F8, isOutput=False),
        "xgv": dp("xgv", [P, NSLOT // P], BF, isOutput=False),
        "gid": dp("gid", [P, NC], FT, isOutput=False),
        "dinvl": dp("dinvl", [P, NC], FT, isOutput=False),
        "xsf": dp("xsf", [P, NC], FT, isOutput=False),
        "xsfn": dp("xsfn", [P, NC], FT, isOutput=False),
        "urep": dp("urep", [P, H], FT, isOutput=False),
        "vrep": dp("vrep", [P, H], FT, isOutput=False),
        "W3s": dp("W3s", [P, 2 * H], BF, isOutput=False),
        "b2rep": dp("b2rep", [P, H], FT, isOutput=False),
        "b3rep": dp("b3rep", [P, H], FT, isOutput=False),
        "Wf1k": dp("Wf1k", [P, 2 * 128], FT, isOutput=False),
        "Wf1c": dp("Wf1c", [1, 128], FT, isOutput=False),
        "bf1rep": dp("bf1rep", [B, 128], FT, isOutput=False),
        "Wf2s": dp("Wf2s", [P, 6], FT, isOutput=False),
        "bf2rep": dp("bf2rep", [B, 6], FT, isOutput=False),
        "svm": dp("svm", [1, B], FT, isOutput=False),
        "invc2": dp("invc2", [P, 2 * B], FT, isOutput=False),
    }
    if not l1_fast:
        pr["W1rep"] = dp("W1rep", [P, H], FT, isOutput=False)
        pr["b1rep"] = dp("b1rep", [P, H], FT, isOutput=False)
        pr["W2s"] = dp("W2s", [P, 2 * H], BF, isOutput=False)
    out_p = dp("out", [B, 6], FT, isOutput=True)

    with tile.TileContext(nc) as tc:
        with (
            tc.tile_pool(name="res", bufs=1) as res,
            tc.tile_pool(name="work", bufs=3) as work,
            tc.tile_pool(name="pp_acc", bufs=2, space="PSUM") as pp_acc,
            tc.tile_pool(name="pp_z", bufs=2, space="PSUM") as pp_z,
            tc.tile_pool(name="pp_t", bufs=2, space="PSUM") as pp_t,
            tc.tile_pool(name="pp_pool", bufs=1, space="PSUM") as pp_pool,
            tc.tile_pool(name="dram", bufs=1, space="DRAM") as dram,
        ):
            zloc = dram.tile([NP, H], F8, name="zloc")
            tabs2 = [dram.tile([RQ, H], F8, addr_space="Shared", name=f"t2q{q}")
                     for q in range(NBUCK)]
            tabs3 = [dram.tile([RQ, H], F8, addr_space="Shared", name=f"t3q{q}")
                     for q in range(NBUCK)]
            ccin = dram.tile([P, 2 * B], FT, name="ccin")
            ccout = dram.tile([P, 2 * B], FT, addr_space="Shared", name="ccout")

            # ---- resident SBUF ----
            sizes = {
                "idxw": ([P, NG * NBUCK * CW], I16),
                "xgv": ([P, NSLOT // P], BF),
                "gid": ([P, NC], FT),
                "dinvl": ([P, NC], FT),
                "xsf": ([P, NC], FT),
                "xsfn": ([P, NC], FT),
                "urep": ([P, H], FT),
                "vrep": ([P, H], FT),
                "W3s": ([P, 2 * H], BF),
                "b2rep": ([P, H], FT),
                "b3rep": ([P, H], FT),
                "Wf1k": ([P, 2 * 128], FT),
                "Wf1c": ([1, 128], FT),
                "bf1rep": ([B, 128], FT),
                "Wf2s": ([P, 6], FT),
                "bf2rep": ([B, 6], FT),
                "svm": ([1, B], FT),
                "invc2": ([P, 2 * B], FT),
            }
            if not l1_fast:
                sizes["W1rep"] = ([P, H], FT)
                sizes["b1rep"] = ([P, H], FT)
                sizes["W2s"] = ([P, 2 * H], BF)
            sb = {}
            for k, (shape, dt) in sizes.items():
                sb[k] = res.tile(shape, dt, name=f"sb_{k}")
                nc.sync.dma_start(sb[k][:], pr[k][:])
            p01_dram = pr["p01"]

            iota64 = res.tile([P, B], FT)
            nc.gpsimd.iota(iota64[:], pattern=[[1, B]], base=0,
                           channel_multiplier=0, allow_small_or_imprecise_dtypes=True)
            ident = res.tile([P, P], FT)
            make_identity(nc, ident[:])

            zsb = res.tile([P, NC, H], F8, name="zsb")

            msgs = [res.tile([P, GT, H], F8, name=f"msgs{i}") for i in range(4)]
            for _m in msgs:
                nc.vector.memset(_m[:], 0.0)

            def build_P(ch):
                """Stream host-precomputed 0/1 one-hot [128 edges, TT*128 dst]."""
                Pt = work.tile([P, TT * P], F8, tag="P")
                lo = ch * TT * P
                nc.sync.dma_start(Pt[:], p01_dram[:, lo:lo + TT * P])
                return Pt

            def h_to_z(hc, W_sb, ch):
                """z chunk = dinv * (h chunk @ W) -> fp8 rows of zsb + zloc."""
                hT = work.tile([P, 2, P], BF, tag="hT")
                for k in range(2):
                    tp = pp_t.tile([P, P], FT, tag="tp", space="PSUM")
                    nc.tensor.transpose(out=tp[:], in_=hc[:, k * P:(k + 1) * P],
                                        identity=ident[:])
                    nc.vector.tensor_copy(hT[:, k, :], tp[:])
                zp = pp_z.tile([P, H], FT, tag="zp", space="PSUM")
                for k in range(2):
                    nc.tensor.matmul(zp[:], lhsT=hT[:, k, :],
                                     rhs=W_sb[:, k * H:(k + 1) * H],
                                     start=(k == 0), stop=(k == 1))
                nc.vector.tensor_scalar(out=zsb[:, ch, :], in0=zp[:],
                                        scalar1=sb["dinvl"][:, ch:ch + 1],
                                        scalar2=None, op0=AL.mult)
                nc.sync.dma_start(zloc[ch * P:(ch + 1) * P, :], zsb[:, ch, :])

            def emit_ag(q, tabs):
                nc.gpsimd.collective_compute(
                    "AllGather", AL.bypass,
                    replica_groups=[list(range(NCORES))],
                    ins=[zloc[q * QS:(q + 1) * QS, :]], outs=[tabs[q].opt()])

            # ================= LAYER 1 =================
            for ch in range(NC):
                Pt = build_P(ch)
                sacc = pp_z.tile([P, 1], FT, tag="zp", space="PSUM")
                for t in range(TT):
                    col = ch * TT + t
                    nc.tensor.matmul(sacc[:], lhsT=Pt[:, t * P:(t + 1) * P],
                                     rhs=sb["xgv"][:, col:col + 1],
                                     start=(t == 0), stop=(t == TT - 1))
                if l1_fast:
                    # z2 = dinv*relu(s)*u + dinv*relu(-s)*v  (b1 == 0)
                    tpos = work.tile([P, 1], FT, tag="tp1")
                    nc.scalar.activation(out=tpos[:], in_=sacc[:], func=AF.Relu,
                                         bias=sb["xsf"][:, ch:ch + 1], scale=1.0)
                    tneg = work.tile([P, 1], FT, tag="tn1")
                    nc.scalar.activation(out=tneg[:], in_=sacc[:], func=AF.Relu,
                                         bias=sb["xsfn"][:, ch:ch + 1], scale=-1.0)
                    acol = work.tile([P, 1], FT, tag="ac1")
                    nc.vector.tensor_tensor(out=acol[:], in0=tpos[:],
                                            in1=sb["dinvl"][:, ch:ch + 1],
                                            op=AL.mult)
                    bcol = work.tile([P, 1], FT, tag="bc1")
                    nc.vector.tensor_tensor(out=bcol[:], in0=tneg[:],
                                            in1=sb["dinvl"][:, ch:ch + 1],
                                            op=AL.mult)
                    tmp = work.tile([P, H], FT, tag="hc")
                    nc.vector.tensor_scalar(out=tmp[:], in0=sb["vrep"][:],
                                            scalar1=bcol[:], scalar2=None,
                                            op0=AL.mult)
                    nc.vector.scalar_tensor_tensor(
                        out=zsb[:, ch, :], in0=sb["urep"][:], scalar=acol[:],
                        in1=tmp[:], op0=AL.mult, op1=AL.add)
                    nc.sync.dma_start(zloc[ch * P:(ch + 1) * P, :], zsb[:, ch, :])
                else:
                    scol = work.tile([P, 1], FT, tag="sc")
                    nc.vector.tensor_tensor(out=scol[:], in0=sacc[:],
                                            in1=sb["xsf"][:, ch:ch + 1], op=AL.add)
                    h1 = work.tile([P, H], FT, tag="hc")
                    nc.vector.scalar_tensor_tensor(
                        out=h1[:], in0=sb["W1rep"][:], scalar=scol[:],
                        in1=sb["b1rep"][:], op0=AL.mult, op1=AL.add)
                    nc.scalar.activation(out=h1[:], in_=h1[:], func=AF.Relu)
                    h_to_z(h1, sb["W2s"], ch)
                if (ch + 1) % QC == 0:
                    emit_ag(ch // QC, tabs2)

            if cut == 1:
                fin0 = work.tile([B, 6], FT, tag="fin")
                nc.vector.memset(fin0[:], 0.0)
                nc.sync.dma_start(out_p[:], fin0[:])

            # ================= LAYERS 2,3 =================
            poolTs = [pp_pool.tile([P, B], FT, tag=f"pool{k}", space="PSUM",
                                   name=f"poolT{k}") for k in range(2)]

            def gather_group(g, tabs, pingpong):
                mt = msgs[pingpong % len(msgs)]
                for b in range(NBUCK):
                    call = g * NBUCK + b
                    nc.gpsimd.dma_gather(
                        mt[:, b * G * TPB:(b + 1) * G * TPB, :],
                        tabs[b][:],
                        sb["idxw"][:, call * CW:(call + 1) * CW],
                        G * BCAP, G * BCAP, H, single_packet=False,
                        queue_num=b % 4)
                return mt

            def msg_layer(tabs, brow, is_last, sub=4, next_tabs=None):
                for g in range(NG):
                    mt = gather_group(g, tabs, g % len(msgs))
                    if sub < 2:
                        continue
                    for j in range(G):
                        ch = g * G + j
                        Pt = build_P(ch)
                        acc = pp_acc.tile([P, H], FT, tag="acc", space="PSUM")
                        i = 0
                        for b in range(NBUCK):
                            for t in range(TPB):
                                nc.tensor.matmul(
                                    acc[:], lhsT=Pt[:, i * P:(i + 1) * P],
                                    rhs=mt[:, (b * G + j) * TPB + t, :],
                                    start=(i == 0),
                                    stop=(i == NBUCK * TPB - 1))
                                i += 1
                        if sub == 2:
                            continue
                        # hc = relu((acc + z_self) * dinv + b)
                        hc = work.tile([P, H], FT, tag="hc")
                        nc.vector.scalar_tensor_tensor(
                            out=hc[:], in0=acc[:],
                            scalar=sb["dinvl"][:, ch:ch + 1],
                            in1=brow[:], op0=AL.mult, op1=AL.add)
                        nc.vector.scalar_tensor_tensor(
                            out=hc[:], in0=zsb[:, ch, :],
                            scalar=sb["dinvl"][:, ch:ch + 1],
                            in1=hc[:], op0=AL.mult, op1=AL.add)
                        nc.scalar.activation(out=hc[:], in_=hc[:], func=AF.Relu)
                        if sub == 3:
                            continue
                        if not is_last:
                            h_to_z(hc, sb["W3s"], ch)
                            if (ch + 1) % QC == 0 and next_tabs is not None:
                                emit_ag(ch // QC, next_tabs)
                        else:
                            oh = work.tile([P, B], FT, tag="oh")
                            nc.vector.tensor_scalar(
                                out=oh[:], in0=iota64[:],
                                scalar1=sb["gid"][:, ch:ch + 1], scalar2=None,
                                op0=AL.is_equal)
                            for k in range(2):
                                nc.tensor.matmul(
                                    poolTs[k][:], lhsT=hc[:, k * P:(k + 1) * P],
                                    rhs=oh[:], start=(ch == 0), stop=(ch == NC - 1))

            if cut != 1:
                msg_layer(tabs2, sb["b2rep"], False,
                          sub=(cut - 20 if 20 < cut < 25 else 4),
                          next_tabs=tabs3)
            if cut == 2 or 20 < cut < 25:
                fin0 = work.tile([B, 6], FT, tag="fin")
                nc.vector.memset(fin0[:], 0.0)
                nc.sync.dma_start(out_p[:], fin0[:])
            if cut == 0:
                msg_layer(tabs3, sb["b3rep"], True)

            if cut == 0:
                # ---- pool epilogue ----
                poolsb = work.tile([P, 2 * B], FT, tag="poolsb")
                for k in range(2):
                    nc.vector.tensor_copy(poolsb[:, k * B:(k + 1) * B], poolTs[k][:])
                nc.sync.dma_start(ccin[:], poolsb[:])
                nc.gpsimd.collective_compute(
                    "AllReduce", AL.add, replica_groups=[list(range(NCORES))],
                    ins=[ccin.opt()], outs=[ccout.opt()])
                pooledT = work.tile([P, 2 * B], FT, tag="pooledT")
                nc.sync.dma_start(pooledT[:], ccout[:])
                nc.vector.tensor_tensor(out=pooledT[:], in0=pooledT[:],
                                        in1=sb["invc2"][:], op=AL.mult)

                # ---- MLP head ----
                o1 = pp_acc.tile([B, 128], FT, tag="acc", space="PSUM")
                pT = pooledT[:].rearrange("p (k b) -> p k b", k=2)
                for k in range(2):
                    nc.tensor.matmul(o1[:], lhsT=pT[:, k, :],
                                     rhs=sb["Wf1k"][:, k * 128:(k + 1) * 128],
                                     start=(k == 0), stop=False)
                nc.tensor.matmul(o1[:], lhsT=sb["svm"][:], rhs=sb["Wf1c"][:],
                                 start=False, stop=True)
                a1 = work.tile([B, 128], FT, tag="a1")
                nc.vector.scalar_tensor_tensor(out=a1[:], in0=o1[:], scalar=1.0,
                                               in1=sb["bf1rep"][:], op0=AL.mult,
                                               op1=AL.add)
                nc.scalar.activation(out=a1[:], in_=a1[:], func=AF.Relu)
                tpa = pp_t.tile([P, B], FT, tag="tp", space="PSUM")
                nc.tensor.transpose(out=tpa[:], in_=a1[:], identity=ident[0:B, 0:B])
                a1T = work.tile([P, B], FT, tag="a1T")
                nc.vector.tensor_copy(a1T[:], tpa[:])
                o2 = pp_z.tile([B, 6], FT, tag="zp", space="PSUM")
                nc.tensor.matmul(o2[:], lhsT=a1T[:], rhs=sb["Wf2s"][:],
                                 start=True, stop=True)
                fin = work.tile([B, 6], FT, tag="fin")
                nc.vector.scalar_tensor_tensor(out=fin[:], in0=o2[:], scalar=1.0,
                                               in1=sb["bf2rep"][:], op0=AL.mult,
                                               op1=AL.add)
                nc.sync.dma_start(out_p[:], fin[:])

    nc.compile()
    return nc


def kernel(x, edge_index, batch, svm_pred,
           W1, b1, W2, b2, W3, b3, Wf1, bf1, Wf2, bf2, **kw):
    from concourse.bass_utils import run_bass_kernel_spmd
    params, in_maps, invc = preprocess(x, edge_index, batch, svm_pred)
    add_weight_inputs(in_maps, params, W1, b1, W2, b2, W3, b3, Wf1, bf1, Wf2, bf2,
                      svm_pred, invc)
    params["l1_fast"] = not np.any(np.asarray(b1))
    if params["l1_fast"]:
        for m in in_maps:
            m.pop("W1rep", None); m.pop("b1rep", None); m.pop("W2s", None)
    nc = build(params)
    res = run_bass_kernel_spmd(nc, in_maps, core_ids=list(range(NCORES)), **kw)
    out = np.asarray(res.results[0]["out"], np.float32)
    if kw:
        return out, res
    return out
